# revision 1
# baseline (speedup 1.0000x reference)
"""Trainium2 Bass kernel: full encoder-decoder transformer decoder layer.

Contract: kernel(**inputs) takes FULL unsharded inputs (see below) and
returns the FULL [B, T, D] float32 output.

Sharding: pure data-parallel over (batch, T-half) -> 8 cores, zero
collectives.  Each core computes its TL=1024 decoder rows end-to-end;
the full-T K/V projections are computed redundantly by the 2 cores
sharing a batch element (~17% extra FLOPs, no cross-core sync).

On-device layout: the token stream is carried TRANSPOSED [D, T] so that
every projection matmul uses a natural weight slice as the stationary
(lhsT) operand and outputs stay transposed.  Attention computes
S^T = K @ Q^T per head, exp on ScalarE (scale=1/sqrt(hd) fused), then
O^T via V-stationary accumulation where an appended ones column yields
the softmax denominators in the same PSUM tile.  LayerNorm reduces over
D (the partition dim) with a ones[128,128] matmul that also broadcasts
the stats to all partitions.
"""

from contextlib import ExitStack

import ml_dtypes
import numpy as np

import concourse.bass as bass
import concourse.mybir as mybir
import concourse.tile as tile
from concourse import bacc
from concourse.bass_utils import run_bass_kernel_spmd

P = 128
HD = 64  # head dim (fixed)
BF = mybir.dt.bfloat16
F32 = mybir.dt.float32
AF = mybir.ActivationFunctionType
ALU = mybir.AluOpType
EPS = 1e-5


# ----------------------------------------------------------------------------
# device program builder
# ----------------------------------------------------------------------------

def build_program(D=1024, H=16, T=2048, TL=1024, S=2048, DFF=4096, loop_n=1):
    """Build the single-core SPMD Bass program.

    D: model dim; H: heads; T: full decoder length (K/V span);
    TL: local query rows; S: encoder length; DFF: ffn dim.
    """
    assert D == H * HD
    KT = D // P            # D tiles
    NKT = T // P           # self-attn k tiles
    NSK = S // P           # cross-attn k tiles
    FT = DFF // P          # ffn tiles
    HP = H // 2            # packed head-pair tiles (== KT)
    assert HP == KT
    QC = min(512, TL)      # query chunk (psum free dim)
    NQ = TL // QC

    nc = bacc.Bacc()

    tens = {}

    def din(name, shape, dtype=BF):
        tens[name] = nc.declare_dram_parameter(name, list(shape), dtype,
                                               isOutput=False)
        return tens[name]

    # streams
    xdT = din("xdT", (KT, P, T))          # decoder stream, transposed, full T
    xqT = din("xqT", (KT, P, TL))         # own-query slice (bf16)
    xres = din("xres", (KT, P, TL), F32)  # own residual slice (f32)
    xeT = din("xeT", (KT, P, S))          # encoder stream, transposed
    # weights (column-permuted on host into head-major [h*64+d] order)
    for nm in ("wq", "wk", "wv", "wo1", "wqc", "wkc", "wvc", "wo2"):
        din(nm, (KT, P, D))
    din("w1", (KT, P, DFF))
    din("w2", (FT, P, D))
    # per-partition bias / LN params (f32)
    for nm in ("bq", "bk", "bo1", "bqc", "bkc", "bo2", "b2f",
               "g1", "be1", "g2", "be2", "g3", "be3"):
        din(nm, (KT, P, 1), F32)
    din("b1f", (FT, P, 1), F32)
    # free-dim bias rows (for V projections, broadcast via DMA)
    din("bv_row", (1, D), F32)
    din("bvc_row", (1, D), F32)

    tens["outT"] = nc.declare_dram_parameter("outT", [KT, P, TL], F32,
                                             isOutput=True)

    # internal DRAM spill for the two residual streams
    tens["res1_spill"] = nc.dram_tensor("res1_spill", [KT, P, TL], F32)
    tens["res2_spill"] = nc.dram_tensor("res2_spill", [KT, P, TL], F32)
    # bounce buffer for softmax reciprocal rows (DRAM-source partition bcast)
    tens["r_bounce"] = nc.dram_tensor("r_bounce", [2, H, NQ, QC], F32)

    cfg = dict(D=D, H=H, T=T, TL=TL, S=S, DFF=DFF, KT=KT, NKT=NKT,
               NSK=NSK, FT=FT, HP=HP, QC=QC, NQ=NQ, tens=tens)

    with tile.TileContext(nc) as tc:
        if loop_n > 1:
            with tc.For_i(0, loop_n, 1) as _i:
                _build(tc, cfg)
        else:
            _build(tc, cfg)

    nc.finalize()
    return nc


def _build(tc, cfg):
    nc = tc.nc
    D, H, T, TL, S, DFF = (cfg["D"], cfg["H"], cfg["T"], cfg["TL"], cfg["S"],
                           cfg["DFF"])
    KT, NKT, NSK, FT, HP, QC, NQ = (cfg["KT"], cfg["NKT"], cfg["NSK"],
                                    cfg["FT"], cfg["HP"], cfg["QC"], cfg["NQ"])

    tens = cfg["tens"]

    def dram(name):
        return tens[name][:]

    ctx = ExitStack()
    # ------------- global pools (never closed before ctx exit) -------------
    const = ctx.enter_context(tc.tile_pool(name="const", bufs=1))
    stream2 = ctx.enter_context(tc.tile_pool(name="stream2", bufs=2))
    # psum: acc [128,512] x4 banks + scores [128,2*QC] x2bufs (4 banks)
    acc = ctx.enter_context(tc.tile_pool(name="acc", bufs=4, space="PSUM"))
    scp = ctx.enter_context(tc.tile_pool(name="scp", bufs=2, space="PSUM"))

    # ---------------- constants ----------------
    ones_bf = const.tile([P, P], BF, tag="ones_bf", name="ones_bf")
    nc.vector.memset(ones_bf[:], 1.0)
    eps_t = const.tile([P, 1], F32, tag="eps_t", name="eps_t")
    nc.vector.memset(eps_t[:], EPS)

    def load_pp(name, n):
        out = []
        src = dram(name)
        for j in range(n):
            tl_ = const.tile([P, 1], F32, tag=f"{name}{j}", name=f"{name}{j}")
            nc.sync.dma_start(out=tl_[:], in_=src[j])
            out.append(tl_)
        return out

    bq = load_pp("bq", KT); bk = load_pp("bk", KT)
    bo1 = load_pp("bo1", KT); bqc = load_pp("bqc", KT)
    bkc = load_pp("bkc", KT); bo2 = load_pp("bo2", KT)
    b1f = load_pp("b1f", FT); b2f = load_pp("b2f", KT)
    g1 = load_pp("g1", KT); be1 = load_pp("be1", KT)
    g2 = load_pp("g2", KT); be2 = load_pp("be2", KT)
    g3 = load_pp("g3", KT); be3 = load_pp("be3", KT)

    def bias_bcast(name):
        tl_ = const.tile([P, D], F32, tag=f"{name}_bc", name=f"{name}_bc")
        src = dram(name)
        bcast_ap = bass.AP(tensor=src.tensor, offset=0, ap=[[0, P], [1, D]])
        nc.gpsimd.dma_start(out=tl_[:], in_=bcast_ap)
        return tl_

    vb_bc = bias_bcast("bv_row")
    vcb_bc = bias_bcast("bvc_row")

    # =====================================================================
    # generic transposed projection:  out^T[F, t] = sum_ki W[ki]^T @ x[ki]
    # fg-outer so each weight chunk is DMA'd exactly once; scoped w pool.
    # =====================================================================
    ACCG = 2

    def projT(wname, x_tiles, nF, Tlen, evict, kt_in=None):
        kt_in = kt_in if kt_in is not None else KT
        C = min(512, Tlen)
        ntc = Tlen // C
        nfj = nF // P
        w = dram(wname)
        with tc.tile_pool(name=f"wp_{wname}", bufs=2) as wp:
            for fg in range((nfj + ACCG - 1) // ACCG):
                js = list(range(fg * ACCG, min((fg + 1) * ACCG, nfj)))
                wts = []
                for ki in range(kt_in):
                    wt = wp.tile([P, len(js) * P], BF, tag=f"k{ki}",
                                 name=f"w_{wname}_{ki}")
                    nc.sync.dma_start(
                        out=wt[:], in_=w[ki][:, js[0] * P:(js[-1] + 1) * P])
                    wts.append(wt)
                for tci in range(ntc):
                    ps = [acc.tile([P, C], F32, tag="acc", name="acc_ps")
                          for _ in js]
                    for ki in range(kt_in):
                        for jj in range(len(js)):
                            nc.tensor.matmul(
                                ps[jj][:],
                                lhsT=wts[ki][:, jj * P:(jj + 1) * P],
                                rhs=x_tiles[ki][:, tci * C:(tci + 1) * C],
                                start=(ki == 0), stop=(ki == kt_in - 1))
                    for jj, fj in enumerate(js):
                        evict(fj, tci, C, ps[jj])

    # =====================================================================
    # attention (self- and cross-)
    # =====================================================================
    def attention(KTt, QTt, Vst, nkt, mergedT, expp, tag, bounce_idx=0,
                  filler=None):
        # Head pairs interleave: even head streams through PE rows 0-63,
        # odd head through rows 64-127 (concurrent row groups).
        rb = tens["r_bounce"]
        with tc.tile_pool(name=f"sm_{tag}", bufs=4) as small:
            for jt in range(HP):
                for qi in range(NQ):
                    avE = acc.tile([P, QC], F32, tag="acc", name="avE")
                    avO = acc.tile([P, QC], F32, tag="acc", name="avO")
                    qsl = slice(qi * QC, (qi + 1) * QC)
                    for kt in range(nkt):
                        sc = scp.tile([P, 2 * QC], F32, tag="sc", name="sc_ps")
                        ksl = slice(kt * P, (kt + 1) * P)
                        nc.tensor.matmul(
                            sc[:, 0:QC],
                            lhsT=KTt[jt][0:HD, ksl], rhs=QTt[jt][0:HD, qsl],
                            start=True, stop=True, tile_position=(0, 0))
                        nc.tensor.matmul(
                            sc[:, QC:2 * QC],
                            lhsT=KTt[jt][HD:P, ksl], rhs=QTt[jt][HD:P, qsl],
                            start=True, stop=True, tile_position=(HD, 0))
                        ex = expp.tile([P, 2 * QC], BF, tag="ex",
                                       name=f"ex_{tag}")
                        nc.scalar.activation(ex[:], sc[:], AF.Exp, scale=0.125)
                        vsl = Vst[kt][:].rearrange("p (h c) -> p h c", c=HD + 1)
                        nc.tensor.matmul(
                            avE[0:HD + 1, :], lhsT=vsl[:, 2 * jt, :],
                            rhs=ex[:, 0:QC],
                            start=(kt == 0), stop=(kt == nkt - 1))
                        nc.tensor.matmul(
                            avO[0:HD + 1, :], lhsT=vsl[:, 2 * jt + 1, :],
                            rhs=ex[:, QC:2 * QC],
                            start=(kt == 0), stop=(kt == nkt - 1))
                    for av, hh in ((avE, 2 * jt), (avO, 2 * jt + 1)):
                        b0 = (hh % 2) * HD
                        r = small.tile([1, QC], F32, tag="recip", name="recip")
                        nc.vector.reciprocal(r[:], av[HD:HD + 1, :])
                        rslot = rb[bounce_idx, hh, qi]
                        nc.sync.dma_start(out=rslot, in_=r[:])
                        bc = small.tile([HD, QC], F32, tag="bcast",
                                        name="bcast")
                        r_bcast = bass.AP(tensor=rslot.tensor,
                                          offset=rslot.offset,
                                          ap=[[0, HD]] + list(rslot.ap[-1:]))
                        nc.gpsimd.dma_start(out=bc[:], in_=r_bcast)
                        nc.vector.tensor_mul(
                            mergedT[jt][b0:b0 + HD, qsl], av[0:HD, :], bc[:])

    # =====================================================================
    # layernorm over D (partition dim), transposed layout.
    # y (f32) overwrites z_f32 in place; optional bf16 copy / spill / output.
    # =====================================================================
    def layer_norm(z_f32, z_bf, g, be, out_bf=None, spill=None, out_dram=None):
        inv_d = 1.0 / D
        with tc.tile_pool(name="lnp", bufs=2) as lnp:
            for tci in range(NQ):
                sl = slice(tci * QC, (tci + 1) * QC)
                psA = acc.tile([P, QC], F32, tag="acc", name="psA")
                psB = acc.tile([P, QC], F32, tag="acc", name="psB")
                for ki in range(KT):
                    nc.tensor.matmul(psA[:], lhsT=ones_bf[:],
                                     rhs=z_bf[ki][:, sl],
                                     start=(ki == 0), stop=(ki == KT - 1))
                zsq = []
                for ki in range(KT):
                    zq = lnp.tile([P, QC], BF, tag=f"zsq{ki % 4}",
                                  name="zsq")
                    nc.vector.tensor_mul(zq[:], z_bf[ki][:, sl],
                                         z_bf[ki][:, sl])
                    zsq.append(zq)
                for ki in range(KT):
                    nc.tensor.matmul(psB[:], lhsT=ones_bf[:], rhs=zsq[ki][:],
                                     start=(ki == 0), stop=(ki == KT - 1))
                mean = lnp.tile([P, QC], F32, tag="mean", name="mean")
                msq = lnp.tile([P, QC], F32, tag="msq", name="msq")
                nc.vector.tensor_scalar_mul(mean[:], psA[:], inv_d)
                nc.vector.tensor_scalar_mul(msq[:], psB[:], inv_d)
                var = lnp.tile([P, QC], F32, tag="var", name="var")
                nc.vector.tensor_mul(var[:], mean[:], mean[:])
                nc.vector.tensor_sub(var[:], msq[:], var[:])
                sd = lnp.tile([P, QC], F32, tag="sd", name="sd")
                nc.scalar.activation(sd[:], var[:], AF.Sqrt, bias=eps_t[:])
                rstd = lnp.tile([P, QC], F32, tag="rstd", name="rstd")
                nc.vector.reciprocal(rstd[:], sd[:])
                mr = lnp.tile([P, QC], F32, tag="mr", name="mr")
                nc.vector.tensor_mul(mr[:], mean[:], rstd[:])
                for ki in range(KT):
                    tmp = lnp.tile([P, QC], F32, tag="lntmp", name="lntmp")
                    nc.vector.tensor_mul(tmp[:], z_f32[ki][:, sl], rstd[:])
                    nc.vector.tensor_sub(tmp[:], tmp[:], mr[:])
                    nc.vector.tensor_scalar(
                        out=z_f32[ki][:, sl], in0=tmp[:],
                        scalar1=g[ki][:], scalar2=be[ki][:],
                        op0=ALU.mult, op1=ALU.add)
                    if out_bf is not None:
                        nc.vector.tensor_copy(out=out_bf[ki][:, sl],
                                              in_=z_f32[ki][:, sl])
                    if spill is not None:
                        nc.sync.dma_start(out=spill[ki][:, sl],
                                          in_=z_f32[ki][:, sl])
                    if out_dram is not None:
                        nc.sync.dma_start(out=out_dram[ki][:, sl],
                                          in_=z_f32[ki][:, sl])

    NVJ = D // 512 if D >= 512 else 1
    VC = min(512, D)

    def v_proj(x_tiles, wv_tiles, Vst, nkt, vbias_bc):
        for kt in range(nkt):
            vsl = Vst[kt][:].rearrange("p (h c) -> p h c", c=HD + 1)
            nc.vector.memset(vsl[:, :, HD:HD + 1], 1.0)
            for vj in range(NVJ):
                ps = acc.tile([P, VC], F32, tag="acc", name="v_ps")
                for ki in range(KT):
                    nc.tensor.matmul(
                        ps[:], lhsT=x_tiles[ki][:, kt * P:(kt + 1) * P],
                        rhs=wv_tiles[ki][:, vj * VC:(vj + 1) * VC],
                        start=(ki == 0), stop=(ki == KT - 1))
                nc.vector.tensor_add(
                    vsl[:, (VC // HD) * vj:(VC // HD) * (vj + 1), 0:HD],
                    ps[:], vbias_bc[:, vj * VC:(vj + 1) * VC])

    # =====================================================================
    # phase structure (pool opens/closes must be LIFO per memory space)
    # =====================================================================
    es_pres = ExitStack()
    pres = es_pres.enter_context(tc.tile_pool(name="pres", bufs=1))
    res_bf = [pres.tile([P, TL], BF, tag=f"rb{k}", name=f"rb{k}")
              for k in range(KT)]

    es_mg = ExitStack()
    pmg = es_mg.enter_context(tc.tile_pool(name="pmg", bufs=1))
    mergedT = [pmg.tile([P, TL], BF, tag=f"mg{j}", name=f"mg{j}")
               for j in range(HP)]

    es_kt = ExitStack()
    pkt = es_kt.enter_context(tc.tile_pool(name="pkt", bufs=1))
    KTt = [pkt.tile([P, T], BF, tag=f"KTt{j}", name=f"KTt{j}")
           for j in range(HP)]
    QTt = [pkt.tile([P, TL], BF, tag=f"QTt{j}", name=f"QTt{j}")
           for j in range(HP)]
    Vst = [pkt.tile([P, H * (HD + 1)], BF, tag=f"Vst{k}", name=f"Vst{k}")
           for k in range(NKT)]

    # ---- Phase 1: self QKV ----
    es_x = ExitStack()
    px = es_x.enter_context(tc.tile_pool(name="px", bufs=1))
    xdT, xqT, vw = [], [], []
    for ki in range(KT):
        tl_ = px.tile([P, T], BF, tag=f"xdT{ki}", name=f"xdT{ki}")
        nc.sync.dma_start(out=tl_[:], in_=dram("xdT")[ki])
        xdT.append(tl_)
    for ki in range(KT):
        tl_ = px.tile([P, TL], BF, tag=f"xqT{ki}", name=f"xqT{ki}")
        nc.sync.dma_start(out=tl_[:], in_=dram("xqT")[ki])
        xqT.append(tl_)
    for ki in range(KT):
        tl_ = px.tile([P, D], BF, tag=f"vw{ki}", name=f"vw{ki}")
        nc.sync.dma_start(out=tl_[:], in_=dram("wv")[ki])
        vw.append(tl_)

    def ev_k(fj, tci, C, ps):
        nc.vector.tensor_scalar_add(
            out=KTt[fj][:, tci * C:(tci + 1) * C], in0=ps[:], scalar1=bk[fj][:])

    def ev_q(fj, tci, C, ps):
        nc.vector.tensor_scalar_add(
            out=QTt[fj][:, tci * C:(tci + 1) * C], in0=ps[:], scalar1=bq[fj][:])

    projT("wk", xdT, D, T, ev_k)
    projT("wq", xqT, D, TL, ev_q)
    v_proj(xdT, vw, Vst, NKT, vb_bc)
    es_x.close()

    # ---- Phase 2: self-attention ----
    es_ex = ExitStack()
    expp = es_ex.enter_context(tc.tile_pool(name="expp", bufs=12))
    attention(KTt, QTt, Vst, NKT, mergedT, expp, "sa", bounce_idx=0)
    es_ex.close()
    es_kt.close()

    # ---- Phase 3: out-proj + residual + LN1 ----
    es_z1 = ExitStack()
    pz1 = es_z1.enter_context(tc.tile_pool(name="pz1", bufs=1))
    z1 = [pz1.tile([P, TL], F32, tag=f"z1_{k}", name=f"z1_{k}")
          for k in range(KT)]
    z1b = [pz1.tile([P, TL], BF, tag=f"z1b{k}", name=f"z1b{k}")
           for k in range(KT)]

    def ev_o1(fj, tci, C, ps):
        xr = stream2.tile([P, C], F32, tag="xr_s", name="xr_s")
        nc.sync.dma_start(out=xr[:],
                          in_=dram("xres")[fj][:, tci * C:(tci + 1) * C])
        sl = slice(tci * C, (tci + 1) * C)
        nc.vector.scalar_tensor_tensor(
            out=z1[fj][:, sl], in0=ps[:], scalar=bo1[fj][:], in1=xr[:],
            op0=ALU.add, op1=ALU.add)
        nc.vector.tensor_copy(out=z1b[fj][:, sl], in_=z1[fj][:, sl])

    projT("wo1", mergedT, D, TL, ev_o1)
    layer_norm(z1, z1b, g1, be1, out_bf=res_bf,
               spill=[dram("res1_spill")[k] for k in range(KT)])
    es_z1.close()
    es_mg.close()

    # ---- Phase 4: cross QKV ----
    es_pres2 = ExitStack()
    pres2 = es_pres2.enter_context(tc.tile_pool(name="pres2", bufs=1))
    res2_bf = [pres2.tile([P, TL], BF, tag=f"r2b{k}", name=f"r2b{k}")
               for k in range(KT)]

    es_mg2 = ExitStack()
    pmg2 = es_mg2.enter_context(tc.tile_pool(name="pmg2", bufs=1))
    mergedC = [pmg2.tile([P, TL], BF, tag=f"mgc{j}", name=f"mgc{j}")
               for j in range(HP)]

    es_ktc = ExitStack()
    pktc = es_ktc.enter_context(tc.tile_pool(name="pktc", bufs=1))
    KTc = [pktc.tile([P, S], BF, tag=f"KTc{j}", name=f"KTc{j}")
           for j in range(HP)]
    QTc = [pktc.tile([P, TL], BF, tag=f"QTc{j}", name=f"QTc{j}")
           for j in range(HP)]
    Vsc = [pktc.tile([P, H * (HD + 1)], BF, tag=f"Vsc{k}", name=f"Vsc{k}")
           for k in range(NSK)]

    es_xe = ExitStack()
    pxe = es_xe.enter_context(tc.tile_pool(name="pxe", bufs=1))
    xeT, vwc = [], []
    for ki in range(KT):
        tl_ = pxe.tile([P, S], BF, tag=f"xeT{ki}", name=f"xeT{ki}")
        nc.sync.dma_start(out=tl_[:], in_=dram("xeT")[ki])
        xeT.append(tl_)
    for ki in range(KT):
        tl_ = pxe.tile([P, D], BF, tag=f"vwc{ki}", name=f"vwc{ki}")
        nc.sync.dma_start(out=tl_[:], in_=dram("wvc")[ki])
        vwc.append(tl_)

    def ev_kc(fj, tci, C, ps):
        nc.vector.tensor_scalar_add(
            out=KTc[fj][:, tci * C:(tci + 1) * C], in0=ps[:],
            scalar1=bkc[fj][:])

    def ev_qc(fj, tci, C, ps):
        nc.vector.tensor_scalar_add(
            out=QTc[fj][:, tci * C:(tci + 1) * C], in0=ps[:],
            scalar1=bqc[fj][:])

    projT("wkc", xeT, D, S, ev_kc)
    projT("wqc", res_bf, D, TL, ev_qc)
    v_proj(xeT, vwc, Vsc, NSK, vcb_bc)
    es_xe.close()

    # ---- Phase 5: cross-attention ----
    es_ex2 = ExitStack()
    expp2 = es_ex2.enter_context(tc.tile_pool(name="expp2", bufs=12))
    attention(KTc, QTc, Vsc, NSK, mergedC, expp2, "ca", bounce_idx=1)
    es_ex2.close()
    es_ktc.close()

    # ---- Phase 6: cross out-proj + residual + LN2 ----
    es_z2 = ExitStack()
    pz2 = es_z2.enter_context(tc.tile_pool(name="pz2", bufs=1))
    z2 = [pz2.tile([P, TL], F32, tag=f"z2_{k}", name=f"z2_{k}")
          for k in range(KT)]
    z2b = [pz2.tile([P, TL], BF, tag=f"z2b{k}", name=f"z2b{k}")
           for k in range(KT)]

    def ev_o2(fj, tci, C, ps):
        xr = stream2.tile([P, C], F32, tag="xr_s", name="xr_s2")
        nc.sync.dma_start(out=xr[:],
                          in_=dram("res1_spill")[fj][:, tci * C:(tci + 1) * C])
        sl = slice(tci * C, (tci + 1) * C)
        nc.vector.scalar_tensor_tensor(
            out=z2[fj][:, sl], in0=ps[:], scalar=bo2[fj][:], in1=xr[:],
            op0=ALU.add, op1=ALU.add)
        nc.vector.tensor_copy(out=z2b[fj][:, sl], in_=z2[fj][:, sl])

    projT("wo2", mergedC, D, TL, ev_o2)
    layer_norm(z2, z2b, g2, be2, out_bf=res2_bf,
               spill=[dram("res2_spill")[k] for k in range(KT)])
    es_z2.close()
    es_mg2.close()

    # ---- Phase 7: FFN + LN3 -> output ----
    es_ff = ExitStack()
    pff = es_ff.enter_context(tc.tile_pool(name="pff", bufs=1))
    hT = [pff.tile([P, TL], BF, tag=f"hT{f}", name=f"hT{f}")
          for f in range(FT)]

    def ev_f1(fj, tci, C, ps):
        nc.vector.tensor_scalar(
            out=hT[fj][:, tci * C:(tci + 1) * C], in0=ps[:],
            scalar1=b1f[fj][:], scalar2=0.0, op0=ALU.add, op1=ALU.max)

    projT("w1", res2_bf, DFF, TL, ev_f1)

    z3 = [pff.tile([P, TL], F32, tag=f"z3_{k}", name=f"z3_{k}")
          for k in range(KT)]
    z3b = [pff.tile([P, TL], BF, tag=f"z3b{k}", name=f"z3b{k}")
           for k in range(KT)]

    def ev_f2(fj, tci, C, ps):
        xr = stream2.tile([P, C], F32, tag="xr_s", name="xr_s3")
        nc.sync.dma_start(out=xr[:],
                          in_=dram("res2_spill")[fj][:, tci * C:(tci + 1) * C])
        sl = slice(tci * C, (tci + 1) * C)
        nc.vector.scalar_tensor_tensor(
            out=z3[fj][:, sl], in0=ps[:], scalar=b2f[fj][:], in1=xr[:],
            op0=ALU.add, op1=ALU.add)
        nc.vector.tensor_copy(out=z3b[fj][:, sl], in_=z3[fj][:, sl])

    projT("w2", hT, D, TL, ev_f2, kt_in=FT)
    layer_norm(z3, z3b, g3, be3,
               out_dram=[dram("outT")[k] for k in range(KT)])
    es_ff.close()
    es_pres2.close()
    es_pres.close()
    ctx.close()


# ----------------------------------------------------------------------------
# host glue
# ----------------------------------------------------------------------------

def _to_bf(a):
    return np.ascontiguousarray(np.asarray(a).astype(ml_dtypes.bfloat16))


def _to_f32(a):
    return np.ascontiguousarray(np.asarray(a).astype(np.float32))


def _prep_weights(inp, D, H, DFF):
    KT = D // P

    def tile_w(w):  # [Din, F] -> [Din//P, P, F]
        return _to_bf(w.reshape(w.shape[0] // P, P, w.shape[1]))

    hidx = np.arange(H)[:, None] * 3 * HD + np.arange(HD)[None, :]
    perm_q = hidx.ravel()
    perm_k = (hidx + HD).ravel()
    perm_v = (hidx + 2 * HD).ravel()
    qkv_w, qkv_b = inp["qkv_w"], inp["qkv_b"]
    kv_w, kv_b = inp["kv_w"], inp["kv_b"]
    h2 = np.arange(H)[:, None] * 2 * HD + np.arange(HD)[None, :]
    perm_kc = h2.ravel()
    perm_vc = (h2 + HD).ravel()

    def pp(b):  # per-partition bias [F] -> [F//P, P, 1] f32
        return _to_f32(np.asarray(b).reshape(-1, P, 1))

    return dict(
        wq=tile_w(qkv_w[:, perm_q]), wk=tile_w(qkv_w[:, perm_k]),
        wv=tile_w(qkv_w[:, perm_v]),
        bq=pp(qkv_b[perm_q]), bk=pp(qkv_b[perm_k]),
        bv_row=_to_f32(qkv_b[perm_v].reshape(1, D)),
        wo1=tile_w(inp["sa_o_w"]), bo1=pp(inp["sa_o_b"]),
        wqc=tile_w(inp["q_w"]), bqc=pp(inp["q_b"]),
        wkc=tile_w(kv_w[:, perm_kc]), bkc=pp(kv_b[perm_kc]),
        wvc=tile_w(kv_w[:, perm_vc]),
        bvc_row=_to_f32(kv_b[perm_vc].reshape(1, D)),
        wo2=tile_w(inp["ca_o_w"]), bo2=pp(inp["ca_o_b"]),
        w1=tile_w(inp["ff_w1"]), b1f=pp(inp["ff_b1"]),
        w2=tile_w(inp["ff_w2"]), b2f=pp(inp["ff_b2"]),
        g1=pp(inp["g1"]), be1=pp(inp["be1"]),
        g2=pp(inp["g2"]), be2=pp(inp["be2"]),
        g3=pp(inp["g3"]), be3=pp(inp["be3"]),
    )


def make_in_maps(inputs, n_cores=8):
    inp = {k: np.asarray(v) for k, v in inputs.items()}
    B, T, D = inp["x_dec"].shape
    S = inp["x_enc"].shape[1]
    DFF = inp["ff_w1"].shape[1]
    H = D // HD
    KT = D // P
    halves = n_cores // B
    TL = T // halves
    shared = _prep_weights(inp, D, H, DFF)
    in_maps = []
    for c in range(n_cores):
        b, half = c // halves, c % halves
        xd = inp["x_dec"][b]                    # [T, D]
        xe = inp["x_enc"][b]                    # [S, D]
        own = xd[half * TL:(half + 1) * TL]     # [TL, D]
        m = dict(shared)
        m["xdT"] = _to_bf(xd.T.reshape(KT, P, T))
        m["xqT"] = _to_bf(own.T.reshape(KT, P, TL))
        m["xres"] = _to_f32(own.T.reshape(KT, P, TL))
        m["xeT"] = _to_bf(xe.T.reshape(KT, P, S))
        in_maps.append(m)
    return in_maps, (B, T, D, TL, S, DFF, H, halves)


def assemble_output(results, meta):
    B, T, D, TL, S, DFF, H, halves = meta
    out = np.empty((B, T, D), np.float32)
    for c, r in enumerate(results):
        b, half = c // halves, c % halves
        yT = np.asarray(r["outT"]).reshape(D, TL)
        out[b, half * TL:(half + 1) * TL] = yT.T
    return out


def kernel(**inputs):
    in_maps, meta = make_in_maps(inputs)
    B, T, D, TL, S, DFF, H, halves = meta
    nc = build_program(D=D, H=H, T=T, TL=TL, S=S, DFF=DFF)
    res = run_bass_kernel_spmd(nc, in_maps, core_ids=list(range(len(in_maps))))
    return assemble_output(res.results, meta)



# revision 68
# speedup vs baseline: 1.0953x; 1.0953x over previous
"""Trainium2 Bass kernel: full encoder-decoder transformer decoder layer.

Contract: kernel(**inputs) takes FULL unsharded inputs (see below) and
returns the FULL [B, T, D] float32 output.

Sharding: pure data-parallel over (batch, T-half) -> 8 cores, zero
collectives.  Each core computes its TL=1024 decoder rows end-to-end;
the full-T K/V projections are computed redundantly by the 2 cores
sharing a batch element (~17% extra FLOPs, no cross-core sync).

On-device layout: the token stream is carried TRANSPOSED [D, T] so that
every projection matmul uses a natural weight slice as the stationary
(lhsT) operand and outputs stay transposed.  Attention computes
S^T = K @ Q^T per head, exp on ScalarE (scale=1/sqrt(hd) fused), then
O^T via V-stationary accumulation where an appended ones column yields
the softmax denominators in the same PSUM tile.  The denominators are
partition-broadcast with a tiny K=2 matmul (pair-select weights), then
a single PSUM reciprocal feeds the normalizing multiplies.  LayerNorm
reduces over D (the partition dim) with a ones[128,128] matmul that
also broadcasts the stats to all partitions.

DMA discipline: every dma_start costs ~0.6us of serialized HWDGE issue
time, so all per-partition bias/LN params travel in ONE packed [128,136]
tensor, weights load as full [128, nF] rows (not per-PSUM-group chunks),
and the big stream loads are emitted first so the PE can start ~10us in.
"""

from contextlib import ExitStack

import ml_dtypes
import numpy as np

import concourse.bass as bass
import concourse.mybir as mybir
import concourse.tile as tile
from concourse import bacc
from concourse.bass_utils import run_bass_kernel_spmd

P = 128
HD = 64  # head dim (fixed)
BF = mybir.dt.bfloat16
F32 = mybir.dt.float32
AF = mybir.ActivationFunctionType
ALU = mybir.AluOpType
EPS = 1e-5

# packed per-partition param column layout (13 KT-sized + b1f FT-sized)
PPK_ORDER = ("bq", "bk", "bo1", "bqc", "bkc", "bo2", "b2f",
             "g1", "be1", "g2", "be2", "g3", "be3")

# (phase_name, last_instruction_index) markers for offline trace analysis
PHASE_MARKS = []


def _mark(nc, name):
    b = nc.m.functions[0].blocks[0]
    idx = int(b.instructions[-1].name.split("-")[1]) if len(
        b.instructions) else 0
    PHASE_MARKS.append((name, idx))


# ----------------------------------------------------------------------------
# device program builder
# ----------------------------------------------------------------------------

def build_program(D=1024, H=16, T=2048, TL=1024, S=2048, DFF=4096, loop_n=1):
    """Build the single-core SPMD Bass program.

    D: model dim; H: heads; T: full decoder length (K/V span);
    TL: local query rows; S: encoder length; DFF: ffn dim.
    """
    assert D == H * HD
    KT = D // P            # D tiles
    NKT = T // P           # self-attn k tiles
    NSK = S // P           # cross-attn k tiles
    FT = DFF // P          # ffn tiles
    HP = H // 2            # packed head-pair tiles (== KT)
    assert HP == KT
    QC = min(512, TL)      # query chunk (psum free dim)
    NQ = TL // QC

    nc = bacc.Bacc()

    tens = {}

    def din(name, shape, dtype=BF):
        tens[name] = nc.declare_dram_parameter(name, list(shape), dtype,
                                               isOutput=False)
        return tens[name]

    # streams.  xdT is ROTATED per core so the core's own TL query columns
    # are always [:, 0:TL] — key order is irrelevant to attention, so the
    # query slice needs no separate tensor.  The residual add uses the
    # bf16 xdT slice directly (error budget allows it).
    din("xdT", (KT, P, T))          # decoder stream, transposed, rotated
    din("xeT", (KT, P, S))          # encoder stream, transposed
    # weights (column-permuted on host into head-major [h*64+d] order)
    for nm in ("wq", "wk", "wv", "wo1", "wqc", "wkc", "wvc", "wo2"):
        din(nm, (KT, P, D))
    din("w1", (KT, P, DFF))
    din("w2", (FT, P, D))
    # ALL per-partition bias / LN params packed into one tensor: one DMA.
    din("ppk", (P, len(PPK_ORDER) * KT + FT), F32)
    # free-dim bias rows (for V projections, broadcast via DMA)
    din("bv_row", (1, D), BF)
    din("bvc_row", (1, D), BF)

    tens["outT"] = nc.declare_dram_parameter("outT", [KT, P, TL], F32,
                                             isOutput=True)

    # internal DRAM spill for the two residual streams (bf16: serves both
    # the residual adds and the next projection's rhs)
    tens["res1bf_spill"] = nc.dram_tensor("res1bf_spill", [KT, P, TL], BF)
    tens["res2bf_spill"] = nc.dram_tensor("res2bf_spill", [KT, P, TL], BF)

    cfg = dict(D=D, H=H, T=T, TL=TL, S=S, DFF=DFF, KT=KT, NKT=NKT,
               NSK=NSK, FT=FT, HP=HP, QC=QC, NQ=NQ, tens=tens)

    with tile.TileContext(nc) as tc:
        if loop_n > 1:
            with tc.For_i(0, loop_n, 1) as _i:
                _build(tc, cfg)
        else:
            _build(tc, cfg)

    nc.finalize()
    return nc


def _build(tc, cfg):
    nc = tc.nc
    D, H, T, TL, S, DFF = (cfg["D"], cfg["H"], cfg["T"], cfg["TL"], cfg["S"],
                           cfg["DFF"])
    KT, NKT, NSK, FT, HP, QC, NQ = (cfg["KT"], cfg["NKT"], cfg["NSK"],
                                    cfg["FT"], cfg["HP"], cfg["QC"], cfg["NQ"])

    tens = cfg["tens"]

    def dram(name):
        return tens[name][:]

    ctx = ExitStack()
    # ------------- global pools (never closed before ctx exit) -------------
    const = ctx.enter_context(tc.tile_pool(name="const", bufs=1))
    # psum: acc [128,512] x4 banks + scores [128,2*QC] x2bufs (4 banks)
    acc = ctx.enter_context(tc.tile_pool(name="acc", bufs=4, space="PSUM"))
    scp = ctx.enter_context(tc.tile_pool(name="scp", bufs=2, space="PSUM"))

    # =====================================================================
    # Phase 1 stream loads FIRST: these gate the very first matmuls, and
    # HWDGE issues serially at ~0.6us/DMA — nothing small goes before them.
    # Long-lived pools are OPENED first (LIFO discipline) but their tiles
    # (pure declarations, no instructions) are created after the DMAs.
    # =====================================================================
    # pxr: bf16 copy of the own residual slice, taken from xdT in ph1 and
    # consumed by the ph3 residual add (outlives px/pkt/pmg)
    es_xr = ExitStack()
    pxr = es_xr.enter_context(tc.tile_pool(name="pxr", bufs=1))
    es_mg = ExitStack()
    pmg = es_mg.enter_context(tc.tile_pool(name="pmg", bufs=1))
    es_kt = ExitStack()
    pkt = es_kt.enter_context(tc.tile_pool(name="pkt", bufs=1))

    es_x = ExitStack()
    px = es_x.enter_context(tc.tile_pool(name="px", bufs=1))
    xdT, vw, wkr = [], [], []
    # interleaved so the first v_proj/wk chains can start ASAP
    for ki in range(KT):
        tl_ = px.tile([P, T], BF, tag=f"xdT{ki}", name=f"xdT{ki}")
        nc.sync.dma_start(out=tl_[:], in_=dram("xdT")[ki])
        xdT.append(tl_)
        tl_ = px.tile([P, D], BF, tag=f"vw{ki}", name=f"vw{ki}")
        nc.sync.dma_start(out=tl_[:], in_=dram("wv")[ki])
        vw.append(tl_)
        tl_ = px.tile([P, D], BF, tag=f"wkr{ki}", name=f"wkr{ki}")
        nc.sync.dma_start(out=tl_[:], in_=dram("wk")[ki])
        wkr.append(tl_)

    # ---------------- constants (one packed DMA for all pp params) --------
    # oD = 1/D in every cell: the LN stats matmuls then produce the mean
    # directly (bf16 holds 2^-10 exactly for D=1024)
    oD = const.tile([P, P], BF, tag="oD", name="oD")
    nc.vector.memset(oD[:], 1.0 / D)
    eps_t = const.tile([P, 1], F32, tag="eps_t", name="eps_t")
    nc.vector.memset(eps_t[:], EPS)
    # ones row for the K=1 softmax-denominator partition-broadcast matmuls
    ones_row = const.tile([1, HD], F32, tag="ones_row", name="ones_row")
    nc.vector.memset(ones_row[:], 1.0)

    NPPK = len(PPK_ORDER) * KT + FT
    ppk = const.tile([P, NPPK], F32, tag="ppk", name="ppk")
    nc.sync.dma_start(out=ppk[:], in_=dram("ppk"))

    def pp_slices(idx, n):
        base = idx * KT if idx < len(PPK_ORDER) else len(PPK_ORDER) * KT
        return [ppk[:, base + j:base + j + 1] for j in range(n)]

    bq = pp_slices(0, KT); bk = pp_slices(1, KT)
    bo1 = pp_slices(2, KT); bqc = pp_slices(3, KT)
    bkc = pp_slices(4, KT); bo2 = pp_slices(5, KT)
    b2f = pp_slices(6, KT)
    g1 = pp_slices(7, KT); be1 = pp_slices(8, KT)
    g2 = pp_slices(9, KT); be2 = pp_slices(10, KT)
    g3 = pp_slices(11, KT); be3 = pp_slices(12, KT)
    b1f = pp_slices(13, FT)

    def bias_bcast(name):
        tl_ = const.tile([P, D], BF, tag=f"{name}_bc", name=f"{name}_bc")
        src = dram(name)
        bcast_ap = bass.AP(tensor=src.tensor, offset=0, ap=[[0, P], [1, D]])
        nc.gpsimd.dma_start(out=tl_[:], in_=bcast_ap)
        return tl_

    vb_bc = bias_bcast("bv_row")
    vcb_bc = bias_bcast("bvc_row")

    # =====================================================================
    # generic transposed projection:  out^T[F, t] = sum_ki W[ki]^T @ x[ki]
    # weights are pre-loaded full rows (w_tiles[ki] = [P, nF] sbuf tile).
    # =====================================================================
    ACCG = 2

    def projT(w_tiles, x_tiles, nF, Tlen, evict, tcis=None):
        # tci-OUTER: all output features for time-chunk 0 complete before
        # chunk 1 starts, so a following layernorm/consumer can pipeline
        # per-chunk instead of waiting for the whole projection.
        kt_in = len(x_tiles)
        C = min(512, Tlen)
        ntc = Tlen // C
        nfj = nF // P
        for tci in (range(ntc) if tcis is None else tcis):
            for fg in range((nfj + ACCG - 1) // ACCG):
                js = list(range(fg * ACCG, min((fg + 1) * ACCG, nfj)))
                ps = [acc.tile([P, C], F32, tag="acc", name="acc_ps")
                      for _ in js]
                for ki in range(kt_in):
                    for jj, fj in enumerate(js):
                        nc.tensor.matmul(
                            ps[jj][:],
                            lhsT=w_tiles[ki][:, fj * P:(fj + 1) * P],
                            rhs=x_tiles[ki][:, tci * C:(tci + 1) * C],
                            start=(ki == 0), stop=(ki == kt_in - 1))
                for jj, fj in enumerate(js):
                    evict(fj, tci, C, ps[jj])

    def load_w_rows(pool, wname, n, cols=None, col0=0):
        out = []
        src = dram(wname)
        for ki in range(n):
            ap = src[ki] if cols is None else src[ki][:, col0:col0 + cols]
            shp = [P, cols if cols is not None else src.shape[-1]]
            wt = pool.tile(shp, BF, tag=f"{wname}{ki}", name=f"{wname}{ki}")
            nc.sync.dma_start(out=wt[:], in_=ap)
            out.append(wt)
        return out

    # =====================================================================
    # attention (self- and cross-)
    # =====================================================================
    def attention(KTt, QTt, Vst, nkt, mergedT, expp, tag):
        # Head pairs interleave: even head streams through PE rows 0-63,
        # odd head through rows 64-127 (concurrent row groups).
        with tc.tile_pool(name=f"sm_{tag}", bufs=2) as small:
            for jt in range(HP):
                for qi in range(NQ):
                    avE = acc.tile([P, QC], F32, tag="acc", name="avE")
                    avO = acc.tile([P, QC], F32, tag="acc", name="avO")
                    qsl = slice(qi * QC, (qi + 1) * QC)
                    for kt in range(nkt):
                        sc = scp.tile([P, 2 * QC], F32, tag="sc", name="sc_ps")
                        ksl = slice(kt * P, (kt + 1) * P)
                        nc.tensor.matmul(
                            sc[:, 0:QC],
                            lhsT=KTt[jt][0:HD, ksl], rhs=QTt[jt][0:HD, qsl],
                            start=True, stop=True, tile_position=(0, 0))
                        nc.tensor.matmul(
                            sc[:, QC:2 * QC],
                            lhsT=KTt[jt][HD:P, ksl], rhs=QTt[jt][HD:P, qsl],
                            start=True, stop=True, tile_position=(HD, 0))
                        ex = expp.tile([P, 2 * QC], BF, tag="ex",
                                       name=f"ex_{tag}")
                        nc.scalar.activation(ex[:], sc[:], AF.Exp, scale=0.125)
                        vsl = Vst[kt][:].rearrange("p (h c) -> p h c", c=HD + 1)
                        nc.tensor.matmul(
                            avE[0:HD + 1, :], lhsT=vsl[:, 2 * jt, :],
                            rhs=ex[:, 0:QC],
                            start=(kt == 0), stop=(kt == nkt - 1))
                        nc.tensor.matmul(
                            avO[0:HD + 1, :], lhsT=vsl[:, 2 * jt + 1, :],
                            rhs=ex[:, QC:2 * QC],
                            start=(kt == 0), stop=(kt == nkt - 1))
                    # softmax denominators: copy the two denom rows into one
                    # partition-0 row, partition-broadcast each half with a
                    # K=1 matmul (col-tiled), reciprocal once on the result.
                    dpair = small.tile([1, 2 * QC], F32, tag="dpair",
                                       name="dpair")
                    nc.vector.tensor_copy(out=dpair[0:1, 0:QC],
                                          in_=avE[HD:HD + 1, :])
                    nc.vector.tensor_copy(out=dpair[0:1, QC:2 * QC],
                                          in_=avO[HD:HD + 1, :])
                    bc = acc.tile([P, QC], F32, tag="acc", name="bc_ps")
                    nc.tensor.matmul(bc[0:HD, :], lhsT=ones_row[:],
                                     rhs=dpair[0:1, 0:QC],
                                     start=True, stop=True,
                                     tile_position=(0, 0))
                    nc.tensor.matmul(bc[HD:P, :], lhsT=ones_row[:],
                                     rhs=dpair[0:1, QC:2 * QC],
                                     start=True, stop=True,
                                     tile_position=(0, HD))
                    rb_t = small.tile([P, QC], F32, tag="rb", name="rb")
                    nc.vector.reciprocal(rb_t[:], bc[:])
                    nc.vector.tensor_mul(
                        mergedT[jt][0:HD, qsl], avE[0:HD, :], rb_t[0:HD, :])
                    nc.vector.tensor_mul(
                        mergedT[jt][HD:P, qsl], avO[0:HD, :], rb_t[HD:P, :])

    # =====================================================================
    # layernorm over D (partition dim), transposed layout.
    # y (f32) overwrites z_f32 in place; optional bf16 copy / spill / output.
    # =====================================================================
    def layer_norm_tci(lnp, z_bf, g, nbe, tci, z_f32=None, spill_bf=None,
                       out_dram=None):
        """One time-chunk of y = (z*rstd)*g - (mean*rstd*g - be).

        Stats come from z_bf via (1/D)-matmuls; Square / Sqrt / the
        per-ki (mr*g - be) run on ScalarE so VectorE only does
        var / reciprocal / mr and 2 ops per ki.  nbe holds -be.
        If z_f32 is None, y overwrites z_bf in place (bf16 residuals).
        """
        sl = slice(tci * QC, (tci + 1) * QC)
        psA = acc.tile([P, QC], F32, tag="acc", name="psA")
        psB = acc.tile([P, QC], F32, tag="acc", name="psB")
        for ki in range(KT):
            nc.tensor.matmul(psA[:], lhsT=oD[:], rhs=z_bf[ki][:, sl],
                             start=(ki == 0), stop=(ki == KT - 1))
        zsq = []
        for ki in range(KT):
            zq = lnp.tile([P, QC], BF, tag=f"zsq{ki % 4}", name="zsq")
            nc.scalar.activation(zq[:], z_bf[ki][:, sl], AF.Square)
            zsq.append(zq)
        for ki in range(KT):
            nc.tensor.matmul(psB[:], lhsT=oD[:], rhs=zsq[ki][:],
                             start=(ki == 0), stop=(ki == KT - 1))
        # psA == mean, psB == mean-square (PSUM)
        mean2 = lnp.tile([P, QC], F32, tag="mean2", name="mean2")
        nc.scalar.activation(mean2[:], psA[:], AF.Square)
        var = lnp.tile([P, QC], F32, tag="var", name="var")
        nc.vector.tensor_sub(var[:], psB[:], mean2[:])
        sd = lnp.tile([P, QC], F32, tag="sd", name="sd")
        nc.scalar.activation(sd[:], var[:], AF.Sqrt, bias=eps_t[:])
        rstd = lnp.tile([P, QC], F32, tag="rstd", name="rstd")
        nc.vector.reciprocal(rstd[:], sd[:])
        mr = lnp.tile([P, QC], F32, tag="mr", name="mr")
        nc.vector.tensor_mul(mr[:], psA[:], rstd[:])
        for ki in range(KT):
            m2 = lnp.tile([P, QC], F32, tag=f"lnm2{ki % 2}", name="lnm2")
            nc.scalar.activation(m2[:], mr[:], AF.Identity,
                                 scale=g[ki], bias=nbe[ki])
            zsrc = z_f32[ki] if z_f32 is not None else z_bf[ki]
            tmp = lnp.tile([P, QC], F32, tag=f"lntmp{ki % 2}", name="lntmp")
            nc.vector.tensor_mul(tmp[:], zsrc[:, sl], rstd[:])
            dst = z_f32[ki] if z_f32 is not None else z_bf[ki]
            nc.vector.scalar_tensor_tensor(
                out=dst[:, sl], in0=tmp[:], scalar=g[ki], in1=m2[:],
                op0=ALU.mult, op1=ALU.subtract)
            if spill_bf is not None:
                nc.sync.dma_start(out=spill_bf[ki][:, sl],
                                  in_=z_bf[ki][:, sl])
            if out_dram is not None:
                nc.sync.dma_start(out=out_dram[ki][:, sl],
                                  in_=z_f32[ki][:, sl])

    NVJ = D // 512 if D >= 512 else 1
    VC = min(512, D)

    def v_proj(x_tiles, wv_tiles, Vst, nkt, vbias_bc):
        for kt in range(nkt):
            vsl = Vst[kt][:].rearrange("p (h c) -> p h c", c=HD + 1)
            nc.vector.memset(vsl[:, :, HD:HD + 1], 1.0)
            for vj in range(NVJ):
                ps = acc.tile([P, VC], F32, tag="acc", name="v_ps")
                for ki in range(len(x_tiles)):
                    nc.tensor.matmul(
                        ps[:], lhsT=x_tiles[ki][:, kt * P:(kt + 1) * P],
                        rhs=wv_tiles[ki][:, vj * VC:(vj + 1) * VC],
                        start=(ki == 0), stop=(ki == len(x_tiles) - 1))
                nc.vector.tensor_add(
                    vsl[:, (VC // HD) * vj:(VC // HD) * (vj + 1), 0:HD],
                    ps[:], vbias_bc[:, vj * VC:(vj + 1) * VC])

    # =====================================================================
    # phase structure.  Pool opens/closes are LIFO per memory space and a
    # pool's SBUF is reserved from open to close, so every pool is scoped
    # to exactly its phase; the bf16 residual streams cross phases via
    # DRAM spills (DMAs are cheap at this count, SBUF is not).
    # =====================================================================
    mergedT = [pmg.tile([P, TL], BF, tag=f"mg{j}", name=f"mg{j}")
               for j in range(HP)]
    KTt = [pkt.tile([P, T], BF, tag=f"KTt{j}", name=f"KTt{j}")
           for j in range(HP)]
    QTt = [pkt.tile([P, TL], BF, tag=f"QTt{j}", name=f"QTt{j}")
           for j in range(HP)]
    Vst = [pkt.tile([P, H * (HD + 1)], BF, tag=f"Vst{k}", name=f"Vst{k}")
           for k in range(NKT)]

    # ---- Phase 1: self QKV ----
    def ev_k(fj, tci, C, ps):
        nc.vector.tensor_scalar_add(
            out=KTt[fj][:, tci * C:(tci + 1) * C], in0=ps[:], scalar1=bk[fj])

    def ev_q(fj, tci, C, ps):
        nc.vector.tensor_scalar_add(
            out=QTt[fj][:, tci * C:(tci + 1) * C], in0=ps[:], scalar1=bq[fj])

    v_proj(xdT, vw, Vst, NKT, vb_bc)
    projT(wkr, xdT, D, T, ev_k)
    # bf16 residual copy (own slice) for the ph3 residual add
    xres_bf = [pxr.tile([P, TL], BF, tag=f"xr{k}", name=f"xr{k}")
               for k in range(KT)]
    for ki in range(KT):
        nc.vector.tensor_copy(out=xres_bf[ki][:], in_=xdT[ki][:, 0:TL])
    # wq rows reuse the vw slots (tag-shared; v_proj is done by then)
    wqr = []
    for ki in range(KT):
        tl_ = px.tile([P, D], BF, tag=f"vw{ki}", name=f"wqr{ki}")
        nc.sync.dma_start(out=tl_[:], in_=dram("wq")[ki])
        wqr.append(tl_)
    # queries are the first TL columns of the rotated xdT
    projT(wqr, xdT, D, TL, ev_q)
    es_x.close()
    _mark(nc, "ph1_selfqkv")

    # ---- Phase 2: self-attention ----
    es_ex = ExitStack()
    expp = es_ex.enter_context(tc.tile_pool(name="expp", bufs=12))
    attention(KTt, QTt, Vst, NKT, mergedT, expp, "sa")
    es_ex.close()
    es_kt.close()
    _mark(nc, "ph2_selfattn")

    # ---- Phase 3: out-proj + residual + LN1 + cross-Q, per-tci ----
    # pmgc/pktc open first (they outlive ph3); cross-Q consumes z1b
    # DIRECTLY (no spill round trip) and interleaves with LN1/out-proj.
    es_mgc = ExitStack()
    pmgc = es_mgc.enter_context(tc.tile_pool(name="pmgc", bufs=1))
    mergedC = [pmgc.tile([P, TL], BF, tag=f"mgc{j}", name=f"mgc{j}")
               for j in range(HP)]

    es_qtc = ExitStack()
    pqtc = es_qtc.enter_context(tc.tile_pool(name="pqtc", bufs=1))
    QTc = [pqtc.tile([P, TL], BF, tag=f"QTc{j}", name=f"QTc{j}")
           for j in range(HP)]

    es_z1 = ExitStack()
    pz1 = es_z1.enter_context(tc.tile_pool(name="pz1", bufs=1))
    lnp1 = es_z1.enter_context(tc.tile_pool(name="lnp1", bufs=1))
    pxq = es_z1.enter_context(tc.tile_pool(name="pxq", bufs=1))
    wo1r = load_w_rows(pz1, "wo1", KT)
    wqcr = load_w_rows(pxq, "wqc", KT)
    z1b = [pz1.tile([P, TL], BF, tag=f"z1b{k}", name=f"z1b{k}")
           for k in range(KT)]

    def ev_o1(fj, tci, C, ps):
        sl = slice(tci * C, (tci + 1) * C)
        nc.vector.scalar_tensor_tensor(
            out=z1b[fj][:, sl], in0=ps[:], scalar=bo1[fj],
            in1=xres_bf[fj][:, sl], op0=ALU.add, op1=ALU.add)

    def ev_qc(fj, tci, C, ps):
        nc.vector.tensor_scalar_add(
            out=QTc[fj][:, tci * C:(tci + 1) * C], in0=ps[:],
            scalar1=bqc[fj])

    # pipeline: out-proj(t) -> LN1(t) -> cross-Q(t), with chunk t+1's PE
    # work running under chunk t's LN (DVE/ACT) work
    r1spill = [dram("res1bf_spill")[k] for k in range(KT)]
    for tci in range(NQ):
        projT(wo1r, mergedT, D, TL, ev_o1, tcis=[tci])
        layer_norm_tci(lnp1, z1b, g1, be1, tci, spill_bf=r1spill)
        projT(wqcr, z1b, D, TL, ev_qc, tcis=[tci])
    es_z1.close()
    # pmg/pxr are dead now but LIFO-pinned below pmgc/pktc; closed at end
    _mark(nc, "ph3_ln1_crossq")

    # ---- Phase 4: cross K/V ----
    es_kvc = ExitStack()
    pkvc = es_kvc.enter_context(tc.tile_pool(name="pkvc", bufs=1))
    KTc = [pkvc.tile([P, S], BF, tag=f"KTc{j}", name=f"KTc{j}")
           for j in range(HP)]
    Vsc = [pkvc.tile([P, H * (HD + 1)], BF, tag=f"Vsc{k}", name=f"Vsc{k}")
           for k in range(NSK)]

    es_xe = ExitStack()
    pxe = es_xe.enter_context(tc.tile_pool(name="pxe", bufs=1))
    xeT = []
    for ki in range(KT):
        tl_ = pxe.tile([P, S], BF, tag=f"xeT{ki}", name=f"xeT{ki}")
        nc.sync.dma_start(out=tl_[:], in_=dram("xeT")[ki])
        xeT.append(tl_)
    wkcr = load_w_rows(pxe, "wkc", KT)

    def ev_kc(fj, tci, C, ps):
        nc.vector.tensor_scalar_add(
            out=KTc[fj][:, tci * C:(tci + 1) * C], in0=ps[:],
            scalar1=bkc[fj])

    projT(wkcr, xeT, D, S, ev_kc)
    # wvc reuses the wkc slots (tag-shared; wkc proj is done by then)
    vwc = []
    for ki in range(KT):
        tl_ = pxe.tile([P, D], BF, tag=f"wkc{ki}", name=f"vwc{ki}")
        nc.sync.dma_start(out=tl_[:], in_=dram("wvc")[ki])
        vwc.append(tl_)
    v_proj(xeT, vwc, Vsc, NSK, vcb_bc)
    es_xe.close()
    _mark(nc, "ph4b_crosskv")

    # ---- Phase 5: cross-attention ----
    es_ex2 = ExitStack()
    expp2 = es_ex2.enter_context(tc.tile_pool(name="expp2", bufs=12))
    attention(KTc, QTc, Vsc, NSK, mergedC, expp2, "ca")
    es_ex2.close()
    es_kvc.close()
    es_qtc.close()
    _mark(nc, "ph5_crossattn")

    # ---- Phase 6: cross out-proj + residual + LN2 (per-tci) ----
    # pffA (FFN weights + residual reloads) opens BEFORE pz2 so its loads
    # prefetch during ph6 and the r2bf chunks stream in right behind LN2.
    es_ffA = ExitStack()
    pffA = es_ffA.enter_context(tc.tile_pool(name="pffA", bufs=1))
    QTR = DFF // 4
    NQF = QTR // P
    w1r0 = []
    for ki in range(KT):
        wt = pffA.tile([P, QTR], BF, tag=f"w1_{ki}", name=f"w1_{ki}_0")
        nc.sync.dma_start(out=wt[:], in_=dram("w1")[ki][:, 0:QTR])
        w1r0.append(wt)
    w2r0 = []
    for ki in range(NQF):
        wt = pffA.tile([P, D], BF, tag=f"w2_{ki}", name=f"w2_{ki}_0")
        nc.sync.dma_start(out=wt[:], in_=dram("w2")[ki])
        w2r0.append(wt)
    r2bf = [pffA.tile([P, TL], BF, tag=f"r2bf{ki}", name=f"r2bf{ki}")
            for ki in range(KT)]

    es_z2 = ExitStack()
    pz2 = es_z2.enter_context(tc.tile_pool(name="pz2", bufs=1))
    wo2r = load_w_rows(pz2, "wo2", KT)
    r1s = []
    for ki in range(KT):
        tl_ = pz2.tile([P, TL], BF, tag=f"r1s{ki}", name=f"r1s{ki}")
        nc.sync.dma_start(out=tl_[:], in_=dram("res1bf_spill")[ki])
        r1s.append(tl_)
    z2b = [pz2.tile([P, TL], BF, tag=f"z2b{k}", name=f"z2b{k}")
           for k in range(KT)]
    lnp2 = es_z2.enter_context(tc.tile_pool(name="lnp2", bufs=1))

    def ev_o2(fj, tci, C, ps):
        sl = slice(tci * C, (tci + 1) * C)
        nc.vector.scalar_tensor_tensor(
            out=z2b[fj][:, sl], in0=ps[:], scalar=bo2[fj], in1=r1s[fj][:, sl],
            op0=ALU.add, op1=ALU.add)

    r2spill = [dram("res2bf_spill")[k] for k in range(KT)]
    for tci in range(NQ):
        projT(wo2r, mergedC, D, TL, ev_o2, tcis=[tci])
        layer_norm_tci(lnp2, z2b, g2, be2, tci, spill_bf=r2spill)
        # stream this chunk straight back for the FFN rhs
        sl = slice(tci * QC, (tci + 1) * QC)
        for ki in range(KT):
            nc.sync.dma_start(out=r2bf[ki][:, sl],
                              in_=dram("res2bf_spill")[ki][:, sl])
    es_z2.close()
    _mark(nc, "ph6_ln2")

    # ---- Phase 7: FFN (DFF quarters, z3 accumulates) + LN3 -> output ----
    es_ff = ExitStack()
    pff = es_ff.enter_context(tc.tile_pool(name="pff", bufs=1))
    z3 = [pff.tile([P, TL], F32, tag=f"z3_{k}", name=f"z3_{k}")
          for k in range(KT)]
    z3b = [None] * KT  # created at q==3, reusing the dead w1 quarter slots

    for q in range(4):
        if q == 0:
            w1r = w1r0
        else:
            w1r = []
            for ki in range(KT):
                wt = pffA.tile([P, QTR], BF, tag=f"w1_{ki}",
                               name=f"w1_{ki}_{q}")
                nc.sync.dma_start(
                    out=wt[:], in_=dram("w1")[ki][:, q * QTR:(q + 1) * QTR])
                w1r.append(wt)
        hTh = [pffA.tile([P, TL], BF, tag=f"hT{f}", name=f"hT{f}_{q}")
               for f in range(NQF)]

        def ev_f1q(fj, tci, C, ps, q=q, hTh=hTh):
            nc.vector.tensor_scalar(
                out=hTh[fj][:, tci * C:(tci + 1) * C], in0=ps[:],
                scalar1=b1f[q * NQF + fj], scalar2=0.0,
                op0=ALU.add, op1=ALU.max)

        projT(w1r, r2bf, QTR, TL, ev_f1q)

        if q == 0:
            w2r = w2r0
        else:
            w2r = []
            for ki in range(NQF):
                wt = pffA.tile([P, D], BF, tag=f"w2_{ki}",
                               name=f"w2_{ki}_{q}")
                nc.sync.dma_start(out=wt[:], in_=dram("w2")[q * NQF + ki])
                w2r.append(wt)
        if q == 3:
            for k in range(KT):
                z3b[k] = pffA.tile([P, TL], BF, tag=f"w1_{k}",
                                   name=f"z3b{k}")

        def ev_f2q(fj, tci, C, ps, q=q):
            sl = slice(tci * C, (tci + 1) * C)
            if q == 0:
                nc.vector.scalar_tensor_tensor(
                    out=z3[fj][:, sl], in0=ps[:], scalar=b2f[fj],
                    in1=r2bf[fj][:, sl], op0=ALU.add, op1=ALU.add)
            elif q < 3:
                nc.vector.tensor_add(z3[fj][:, sl], z3[fj][:, sl], ps[:])
            else:
                nc.vector.tensor_add(z3[fj][:, sl], z3[fj][:, sl], ps[:])
                nc.vector.tensor_copy(out=z3b[fj][:, sl],
                                      in_=z3[fj][:, sl])

        if q < 3:
            projT(w2r, hTh, D, TL, ev_f2q)
        else:
            # last quarter: interleave LN3 per time-chunk with the final
            # w2 accumulation so the output epilogue overlaps PE work
            lnp3 = es_ff.enter_context(tc.tile_pool(name="lnp3", bufs=2))
            outd = [dram("outT")[k] for k in range(KT)]
            for tci in range(NQ):
                projT(w2r, hTh, D, TL, ev_f2q, tcis=[tci])
                layer_norm_tci(lnp3, z3b, g3, be3, tci, z_f32=z3,
                               out_dram=outd)

    es_ff.close()
    es_ffA.close()
    es_mgc.close()
    es_mg.close()
    es_xr.close()
    ctx.close()
    _mark(nc, "ph7_ffn")


# ----------------------------------------------------------------------------
# host glue
# ----------------------------------------------------------------------------

def _to_bf(a):
    return np.ascontiguousarray(np.asarray(a).astype(ml_dtypes.bfloat16))


def _to_f32(a):
    return np.ascontiguousarray(np.asarray(a).astype(np.float32))


def _prep_weights(inp, D, H, DFF):
    KT = D // P
    FT = DFF // P

    def tile_w(w):  # [Din, F] -> [Din//P, P, F]
        return _to_bf(w.reshape(w.shape[0] // P, P, w.shape[1]))

    hidx = np.arange(H)[:, None] * 3 * HD + np.arange(HD)[None, :]
    perm_q = hidx.ravel()
    perm_k = (hidx + HD).ravel()
    perm_v = (hidx + 2 * HD).ravel()
    qkv_w, qkv_b = inp["qkv_w"], inp["qkv_b"]
    kv_w, kv_b = inp["kv_w"], inp["kv_b"]
    h2 = np.arange(H)[:, None] * 2 * HD + np.arange(HD)[None, :]
    perm_kc = h2.ravel()
    perm_vc = (h2 + HD).ravel()

    # be1/2/3 columns hold NEGATED beta: the device computes
    # y = (z*rstd)*g - (mean*rstd*g + (-be)) via ACT Copy(scale=g, bias=-be)
    pp_vals = dict(
        bq=qkv_b[perm_q], bk=qkv_b[perm_k],
        bo1=inp["sa_o_b"], bqc=inp["q_b"], bkc=kv_b[perm_kc],
        bo2=inp["ca_o_b"], b2f=inp["ff_b2"],
        g1=inp["g1"], be1=-np.asarray(inp["be1"]),
        g2=inp["g2"], be2=-np.asarray(inp["be2"]),
        g3=inp["g3"], be3=-np.asarray(inp["be3"]),
    )
    ppk = np.zeros((P, len(PPK_ORDER) * KT + FT), np.float32)
    for i, nm in enumerate(PPK_ORDER):
        ppk[:, i * KT:(i + 1) * KT] = \
            np.asarray(pp_vals[nm]).reshape(KT, P).T
    ppk[:, len(PPK_ORDER) * KT:] = \
        np.asarray(inp["ff_b1"]).reshape(FT, P).T

    return dict(
        wq=tile_w(qkv_w[:, perm_q]), wk=tile_w(qkv_w[:, perm_k]),
        wv=tile_w(qkv_w[:, perm_v]),
        bv_row=_to_bf(qkv_b[perm_v].reshape(1, D)),
        wo1=tile_w(inp["sa_o_w"]),
        wqc=tile_w(inp["q_w"]),
        wkc=tile_w(kv_w[:, perm_kc]),
        wvc=tile_w(kv_w[:, perm_vc]),
        bvc_row=_to_bf(kv_b[perm_vc].reshape(1, D)),
        wo2=tile_w(inp["ca_o_w"]),
        w1=tile_w(inp["ff_w1"]),
        w2=tile_w(inp["ff_w2"]),
        ppk=_to_f32(ppk),
    )


def make_in_maps(inputs, n_cores=8):
    inp = {k: np.asarray(v) for k, v in inputs.items()}
    B, T, D = inp["x_dec"].shape
    S = inp["x_enc"].shape[1]
    DFF = inp["ff_w1"].shape[1]
    H = D // HD
    KT = D // P
    halves = n_cores // B
    TL = T // halves
    shared = _prep_weights(inp, D, H, DFF)
    in_maps = []
    for c in range(n_cores):
        b, half = c // halves, c % halves
        xd = inp["x_dec"][b]                    # [T, D]
        xe = inp["x_enc"][b]                    # [S, D]
        # rotate so this core's own TL query rows come first (key order is
        # irrelevant to attention as long as K and V agree)
        xrot = np.concatenate([xd[half * TL:], xd[:half * TL]], axis=0)
        m = dict(shared)
        m["xdT"] = _to_bf(xrot.T.reshape(KT, P, T))
        m["xeT"] = _to_bf(xe.T.reshape(KT, P, S))
        in_maps.append(m)
    return in_maps, (B, T, D, TL, S, DFF, H, halves)


def assemble_output(results, meta):
    B, T, D, TL, S, DFF, H, halves = meta
    out = np.empty((B, T, D), np.float32)
    for c, r in enumerate(results):
        b, half = c // halves, c % halves
        yT = np.asarray(r["outT"]).reshape(D, TL)
        out[b, half * TL:(half + 1) * TL] = yT.T
    return out


def kernel(**inputs):
    in_maps, meta = make_in_maps(inputs)
    B, T, D, TL, S, DFF, H, halves = meta
    nc = build_program(D=D, H=H, T=T, TL=TL, S=S, DFF=DFF)
    res = run_bass_kernel_spmd(nc, in_maps, core_ids=list(range(len(in_maps))))
    return assemble_output(res.results, meta)
